# revision 9
# baseline (speedup 1.0000x reference)
"""Multi-head causal attention (B=1, S=4096, D=768, H=12) on 8 trn2 NeuronCores.

Sharding: tensor-parallel over heads + causal-balanced split of the query range.
  - cores 0-5 ("late"):  2 heads each, q in [1792, 4096), k in [0, 4096)
  - cores 6-7 ("early"): 6 heads each, q in [0, 1792),  k in [0, 1792)

v2: all-bf16 datapath (fp32 PSUM accumulation) to avoid the fp32r tensor-engine
power throttle; k-outer flash attention with wide psum score tiles (one exp per
k-tile); V bias folded into the host-side output bias; batched 3-D DMAs; bf16
partial outputs summed on host in fp32.

Each core computes qkv projections for its heads, causal softmax(QK^T)V in a
transposed layout (seq on the free axis), and a partial out-projection
(contraction over its heads' dims).  The host sums partials, adds the
effective bias (b_out + W_out @ b_v), and transposes back.
"""

import os
import sys
import threading

sys.path.insert(0, "/opt/trn_rl_repo")

import numpy as np
import ml_dtypes

import concourse.bass as bass
import concourse.mybir as mybir
import concourse.tile as tile
from concourse import bacc
from concourse.masks import make_identity

# ---------------------------------------------------------------- constants
B, S, D, H, DH = 1, 4096, 768, 12, 64
SCALE = DH ** -0.5
P = 128          # sbuf partitions
KT = 128         # key tile (partition axis of scores)
SPLIT = 1792     # early/late query split point
DT = mybir.dt.float32
BF = mybir.dt.bfloat16

CLASSES = {
    # name: (n_pairs, q0, q1, k_len)
    "late": (1, SPLIT, S, S),
    "early": (3, 0, SPLIT, SPLIT),
}


def _chunks512(lo, hi):
    """[lo, hi) split at multiples of 512 (psum bank boundaries)."""
    out = []
    a = lo
    while a < hi:
        b = min((a // 512 + 1) * 512, hi)
        out.append((a, b))
        a = b
    return out


def build_module(cls, debug_taps=False):
    n_pairs, q0, q1, k_len = CLASSES[cls]
    f_c = 128 * n_pairs          # per-core feature width of each projection
    q_len = q1 - q0
    n_kt = k_len // KT           # ktiles of the core's k-support
    n_dt = D // P                # 6 contraction tiles for the projections
    # q-range segments sized to psum-bank multiples (512 fp32 cols):
    # ssc[128,SEG] (2 banks) x2 + acc[65,SEG] (2 banks) x2 = 8 banks
    n_seg = 3 if q_len % 3 == 0 else 2
    SEG = q_len // n_seg
    segs = [(i * SEG, (i + 1) * SEG) for i in range(n_seg)]

    nc = bacc.Bacc("TRN2", target_bir_lowering=False, debug=False,
                   enable_asserts=True, num_devices=1)

    xT = nc.dram_tensor("xT", [D, k_len], BF, kind="ExternalInput")
    wqT = nc.dram_tensor("wqT", [D, f_c], BF, kind="ExternalInput")
    wkT = nc.dram_tensor("wkT", [D, f_c], BF, kind="ExternalInput")
    wvT = nc.dram_tensor("wvT", [D, f_c], BF, kind="ExternalInput")
    bq = nc.dram_tensor("bq", [n_pairs * P, 1], DT, kind="ExternalInput")
    woT = nc.dram_tensor("woT", [f_c, D], BF, kind="ExternalInput")
    dmask = nc.dram_tensor("dmask", [P, P], BF, kind="ExternalInput")
    yT = nc.dram_tensor("yT", [D, q_len], BF, kind="ExternalOutput")
    if debug_taps:
        qTd = nc.dram_tensor("qTd", [n_pairs, P, q_len], BF, kind="ExternalOutput")
        kTd = nc.dram_tensor("kTd", [n_pairs, P, k_len], BF, kind="ExternalOutput")
        vTd = nc.dram_tensor("vTd", [n_pairs, P, k_len], BF, kind="ExternalOutput")
        vktd = nc.dram_tensor("vktd", [n_pairs, P, 132 * (k_len // KT)], BF,
                              kind="ExternalOutput")
        aTd = nc.dram_tensor("aTd", [n_pairs, P, q_len], BF, kind="ExternalOutput")

    with tile.TileContext(nc) as tc:
        with (
            tc.tile_pool(name="w", bufs=1) as sb_w,
            tc.tile_pool(name="x", bufs=2) as sb_x,
            tc.tile_pool(name="persist", bufs=1) as sb_per,
            tc.tile_pool(name="exp", bufs=3) as sb_exp,
            tc.tile_pool(name="rn", bufs=2) as sb_rn,
            tc.tile_pool(name="yout", bufs=2) as sb_y,
        ):
            # ---------------- constants / weights to SBUF
            wq_sb = sb_w.tile([P, n_dt, f_c], BF, tag="wq")
            nc.sync.dma_start(out=wq_sb, in_=wqT.rearrange("(t p) f -> p t f", p=P))
            wk_sb = sb_w.tile([P, n_dt, f_c], BF, tag="wk")
            nc.sync.dma_start(out=wk_sb, in_=wkT.rearrange("(t p) f -> p t f", p=P))
            wv_sb = sb_w.tile([P, n_dt, f_c], BF, tag="wv")
            nc.sync.dma_start(out=wv_sb, in_=wvT.rearrange("(t p) f -> p t f", p=P))
            bq_sb = sb_w.tile([P, n_pairs], DT, tag="bq")
            nc.sync.dma_start(out=bq_sb, in_=bq.rearrange("(n p) o -> p (n o)", p=P))
            wo_sb = sb_w.tile([P, n_pairs, n_dt, P], BF, tag="wo")
            nc.sync.dma_start(
                out=wo_sb,
                in_=woT.rearrange("(n p) (t m) -> p n t m", p=P, m=P))
            dmask_sb = sb_w.tile([P, P], BF, tag="dmask")
            nc.sync.dma_start(out=dmask_sb, in_=dmask.ap())
            ident = sb_w.tile([P, P], BF, tag="ident")
            make_identity(nc, ident)

            # ---------------- persistent activations (head pair packed on
            # partitions: head A rows 0-63, head B rows 64-127)
            qT = [sb_per.tile([P, q_len], BF, name=f"qT{p}", tag=f"qT{p}")
                  for p in range(n_pairs)]
            kT = [sb_per.tile([P, k_len], BF, name=f"kT{p}", tag=f"kT{p}")
                  for p in range(n_pairs)]
            vT = [sb_per.tile([P, k_len], BF, name=f"vT{p}", tag=f"vT{p}")
                  for p in range(n_pairs)]
            # per ktile: [V_A | 1 | pad | V_B | 1 | pad], k on partitions
            vkt = [sb_per.tile([P, n_kt * 132], BF, name=f"vkt{p}",
                               tag=f"vkt{p}") for p in range(n_pairs)]
            aT = [sb_per.tile([P, q_len], BF, name=f"aT{p}", tag=f"aT{p}")
                  for p in range(n_pairs)]

            # ---------------- phase 1: projections (qkvT = W^T-slices @ xT)
            chunks = []
            s0 = 0
            while s0 < k_len:
                w = min(512, k_len - s0)
                chunks.append((s0, w))
                s0 += w
            with tc.tile_pool(name="psP", bufs=2, space="PSUM") as ps_proj:
                for (s0, w) in chunks:
                    xt = sb_x.tile([P, n_dt, 512], BF, tag="xt")
                    nc.sync.dma_start(
                        out=xt[:, :, :w],
                        in_=xT.rearrange("(t p) s -> p t s", p=P)[:, :, s0:s0 + w])
                    for p in range(n_pairs):
                        ps = ps_proj.tile([P, 3, 512], DT, tag="ps")
                        do_q = s0 + w > q0  # chunk overlaps the q-range
                        for dti in range(n_dt):
                            first, last = dti == 0, dti == n_dt - 1
                            if do_q:
                                nc.tensor.matmul(
                                    ps[:, 0, :w],
                                    wq_sb[:, dti, p * P:(p + 1) * P],
                                    xt[:, dti, :w], start=first, stop=last)
                            nc.tensor.matmul(
                                ps[:, 1, :w],
                                wk_sb[:, dti, p * P:(p + 1) * P],
                                xt[:, dti, :w], start=first, stop=last)
                            nc.tensor.matmul(
                                ps[:, 2, :w],
                                wv_sb[:, dti, p * P:(p + 1) * P],
                                xt[:, dti, :w], start=first, stop=last)
                        if do_q:  # q += bias, into persistent qT (q-range cols)
                            lo = max(s0, q0)
                            nc.vector.tensor_scalar_add(
                                qT[p][:, lo - q0:s0 + w - q0],
                                ps[:, 0, lo - s0:w], bq_sb[:, p:p + 1])
                        nc.vector.tensor_copy(kT[p][:, s0:s0 + w], ps[:, 1, :w])
                        nc.vector.tensor_copy(vT[p][:, s0:s0 + w], ps[:, 2, :w])

            # ---------------- phase 2: V -> [k, dh] tiles (+ ones column)
            with tc.tile_pool(name="psT", bufs=2, space="PSUM") as ps_tr:
                for p in range(n_pairs):
                    # ones columns (denominator row of the AV matmul)
                    nc.vector.memset(
                        vkt[p].rearrange("p (k a c) -> p k a c",
                                         k=n_kt, c=66)[:, :, :, 64], 1.0)
                    for k in range(n_kt):
                        pt = ps_tr.tile([P, P], BF, tag="pt", name="pt")
                        nc.tensor.transpose(
                            pt, vT[p][:, k * KT:(k + 1) * KT], ident)
                        nc.vector.tensor_copy(
                            vkt[p][:, k * 132:(k + 1) * 132].rearrange(
                                "p (a c) -> p a c", a=2)[:, :, 0:64],
                            pt.rearrange("p (a c) -> p a c", a=2))

            # ---------------- phase 3: attention (k-outer flash, per
            # (pair, q-half, head); scores kept transposed: k on partitions)
            with (
                tc.tile_pool(name="psS", bufs=2, space="PSUM") as ps_sc,
                tc.tile_pool(name="psA", bufs=2, space="PSUM") as ps_acc,
            ):
                for p in range(n_pairs):
                    for (h0, h1) in segs:
                        Wd = h1 - h0
                        base = q0 + h0          # global q start of this half
                        kmax = (q0 + h1) // KT  # causal ktile count
                        for hi in (0, 1):
                            hs = slice(hi * 64, (hi + 1) * 64)
                            acc = ps_acc.tile([65, SEG], DT, tag="acc")
                            for k in range(kmax):
                                es = max(0, k * KT - base)
                                ssc = ps_sc.tile([P, SEG], DT, tag="ssc")
                                for (a, b) in _chunks512(es, Wd):
                                    nc.tensor.matmul(
                                        ssc[:, a:b],
                                        kT[p][hs, k * KT:(k + 1) * KT],
                                        qT[p][hs, h0 + a:h0 + b],
                                        start=True, stop=True)
                                ex = sb_exp.tile([P, SEG], BF, tag="ex")
                                nc.scalar.activation(
                                    ex[:, es:Wd], ssc[:, es:Wd],
                                    mybir.ActivationFunctionType.Exp,
                                    scale=SCALE)
                                if k * KT >= base:  # diagonal block: mask
                                    nc.vector.tensor_mul(
                                        ex[:, es:es + KT],
                                        ex[:, es:es + KT], dmask_sb)
                                vsl = vkt[p][:, k * 132 + hi * 66:
                                             k * 132 + hi * 66 + 65]
                                for (a, b) in _chunks512(es, Wd):
                                    lastk = (base + b - 1) // KT
                                    nc.tensor.matmul(
                                        acc[:, a:b], vsl, ex[:, a:b],
                                        start=(k == 0), stop=(k == lastk))
                            # normalize: a = num * (1/den), den broadcast
                            rr = sb_rn.tile([1, SEG], DT, tag="rr")
                            nc.vector.reciprocal(rr[:, :Wd], acc[64:65, :Wd])
                            rb = sb_rn.tile([64, SEG], DT, tag="rb")
                            nc.gpsimd.partition_broadcast(
                                rb[:, :Wd], rr[:, :Wd])
                            nc.vector.tensor_mul(
                                aT[p][hs, h0:h1], acc[0:64, :Wd], rb[:, :Wd])

            # ---------------- phase 4: out-projection (partial, this core's
            # head dims only; host sums partials)
            with tc.tile_pool(name="psY", bufs=2, space="PSUM") as ps_y:
                c0 = 0
                while c0 < q_len:
                    w = min(512, q_len - c0)
                    ysb = sb_y.tile([P, n_dt, 512], BF, tag="ysb")
                    for mt in range(n_dt):
                        psy = ps_y.tile([P, 512], DT, tag="psy")
                        for p in range(n_pairs):
                            nc.tensor.matmul(
                                psy[:, :w], wo_sb[:, p, mt, :],
                                aT[p][:, c0:c0 + w],
                                start=(p == 0), stop=(p == n_pairs - 1))
                        nc.vector.tensor_copy(ysb[:, mt, :w], psy[:, :w])
                    nc.sync.dma_start(
                        out=yT.rearrange("(t p) s -> p t s", p=P)[:, :, c0:c0 + w],
                        in_=ysb[:, :, :w])
                    c0 += w

            if debug_taps:
                for p in range(n_pairs):
                    nc.sync.dma_start(out=qTd[p], in_=qT[p])
                    nc.sync.dma_start(out=kTd[p], in_=kT[p])
                    nc.sync.dma_start(out=vTd[p], in_=vT[p])
                    nc.sync.dma_start(out=vktd[p], in_=vkt[p])
                    nc.sync.dma_start(out=aTd[p], in_=aT[p])

    nc.compile()
    return nc


# ---------------------------------------------------------------- host side
def _head_cols(heads):
    """column indices into a [*, 768] head-blocked axis for the given heads"""
    return np.concatenate([np.arange(h * DH, (h + 1) * DH) for h in heads])


def _bf(a):
    return np.ascontiguousarray(a.astype(ml_dtypes.bfloat16))


def make_in_maps(x, W_in, b_in, W_out):
    """Returns (late_in_maps[6], early_in_maps[2])."""
    xT = np.ascontiguousarray(x.reshape(S, D).T)          # [768, 4096]
    WT = np.ascontiguousarray(W_in.T)                     # [768, 2304]
    WoT = np.ascontiguousarray(W_out.T)                   # [768, 768]

    dm = _bf(np.triu(np.ones((P, P), np.float32)))        # k <= q (diag tile)
    xT_bf = _bf(xT)

    def core_inputs(heads, cls):
        _, q0, q1, k_len = CLASSES[cls]
        cols = _head_cols(heads)
        return {
            "xT": np.ascontiguousarray(xT_bf[:, :k_len]),
            "wqT": _bf(WT[:, cols]),
            "wkT": _bf(WT[:, 768 + cols]),
            "wvT": _bf(WT[:, 1536 + cols]),
            "bq": np.ascontiguousarray(b_in[cols][:, None]).astype(np.float32),
            "woT": _bf(WoT[cols, :]),
            "dmask": dm,
        }

    late = [core_inputs([2 * c, 2 * c + 1], "late") for c in range(6)]
    early = [core_inputs(list(range(6 * e, 6 * e + 6)), "early")
             for e in range(2)]
    return late, early


def effective_bias(b_in, W_out, b_out):
    """b_out + W_out @ b_v  (V bias folded out of the device kernel)."""
    return b_out + W_out @ b_in[1536:2304]


def assemble_output(late_res, early_res, b_eff):
    yT = np.zeros((D, S), np.float32)
    for r in late_res:
        yT[:, SPLIT:] += np.asarray(r["yT"], dtype=np.float32)
    for r in early_res:
        yT[:, :SPLIT] += np.asarray(r["yT"], dtype=np.float32)
    y = yT.T + b_eff[None, :]
    return y.reshape(B, S, D).astype(np.float32)


# ------------------------------------------- pjrt runner (explicit devices)
def _run_group(nc, in_maps, devices):
    """run_bass_via_pjrt equivalent on an explicit device subset."""
    import jax
    from jax.sharding import Mesh, PartitionSpec
    from jax.experimental.shard_map import shard_map
    from concourse import bass2jax
    from concourse.bass2jax import _bass_exec_p, partition_id_tensor

    bass2jax.install_neuronx_cc_hook()
    n_cores = len(in_maps)
    partition_name = (nc.partition_id_tensor.name
                      if nc.partition_id_tensor else None)

    in_names, out_names, out_avals, zero_outs = [], [], [], []
    for alloc in nc.m.functions[0].allocations:
        if not isinstance(alloc, mybir.MemoryLocationSet):
            continue
        name = alloc.memorylocations[0].name
        if alloc.kind == "ExternalInput":
            if name != partition_name:
                in_names.append(name)
        elif alloc.kind == "ExternalOutput":
            shape = tuple(alloc.tensor_shape)
            dtype = mybir.dt.np(alloc.dtype)
            out_names.append(name)
            out_avals.append(jax.core.ShapedArray(shape, dtype))
            zero_outs.append(np.zeros(shape, dtype))
    n_params = len(in_names)
    n_outs = len(out_avals)
    in_names = in_names + out_names
    if partition_name is not None:
        in_names.append(partition_name)
    donate = tuple(range(n_params, n_params + n_outs))

    def _body(*args):
        operands = list(args)
        if partition_name is not None:
            operands.append(partition_id_tensor())
        outs = _bass_exec_p.bind(
            *operands,
            out_avals=tuple(out_avals),
            in_names=tuple(in_names),
            out_names=tuple(out_names),
            lowering_input_output_aliases=(),
            sim_require_finite=True,
            sim_require_nnan=True,
            nc=nc,
        )
        return tuple(outs)

    per_core = [[np.asarray(m[name]) for name in in_names[:n_params]]
                for m in in_maps]
    if n_cores == 1:
        out_arrs = jax.jit(_body, donate_argnums=donate, keep_unused=True)(
            *per_core[0], *zero_outs)
        return [{n: np.asarray(out_arrs[i]) for i, n in enumerate(out_names)}]

    mesh = Mesh(np.asarray(devices), ("core",))
    in_specs = (PartitionSpec("core"),) * (n_params + n_outs)
    out_specs = (PartitionSpec("core"),) * len(out_names)
    sharded = jax.jit(
        shard_map(_body, mesh=mesh, in_specs=in_specs, out_specs=out_specs,
                  check_rep=False),
        donate_argnums=donate, keep_unused=True)
    concat_in = [np.concatenate([per_core[c][i] for c in range(n_cores)],
                                axis=0) for i in range(n_params)]
    concat_zeros = [np.zeros((n_cores * z.shape[0], *z.shape[1:]), z.dtype)
                    for z in zero_outs]
    out_arrs = sharded(*concat_in, *concat_zeros)
    return [
        {n: np.asarray(out_arrs[i]).reshape(n_cores, *out_avals[i].shape)[c]
         for i, n in enumerate(out_names)}
        for c in range(n_cores)
    ]


_MODULES = {}
_WARM = set()


def _get_module(cls):
    if cls not in _MODULES:
        _MODULES[cls] = build_module(cls)
    return _MODULES[cls]


def kernel(x, W_in, b_in, W_out, b_out):
    import jax
    x = np.asarray(x, np.float32)
    W_in = np.asarray(W_in, np.float32)
    b_in = np.asarray(b_in, np.float32)
    W_out = np.asarray(W_out, np.float32)
    b_out = np.asarray(b_out, np.float32)

    late_maps, early_maps = make_in_maps(x, W_in, b_in, W_out)
    b_eff = effective_bias(b_in, W_out, b_out)
    nc_late = _get_module("late")
    nc_early = _get_module("early")

    devs = jax.devices()
    results = {}
    errs = {}

    def run(tag, nc, maps, devices):
        try:
            results[tag] = _run_group(nc, maps, devices)
        except Exception as e:  # noqa: BLE001
            errs[tag] = e

    # first call per module compiles (serialize those); afterwards the two
    # device groups (cores 0-5 and 6-7) execute concurrently
    t1 = threading.Thread(target=run, args=("late", nc_late, late_maps, devs[0:6]))
    t2 = threading.Thread(target=run, args=("early", nc_early, early_maps, devs[6:8]))
    if not _WARM:
        t1.start(); t1.join()
        t2.start(); t2.join()
        _WARM.add(True)
    else:
        t1.start(); t2.start()
        t1.join(); t2.join()
    if errs:
        raise next(iter(errs.values()))

    return assemble_output(results["late"], results["early"], b_eff)
